# revision 34
# baseline (speedup 1.0000x reference)
"""MoE decoder kernel for Trainium2 (8 NeuronCores, expert-parallel).

Strategy
--------
Host (numpy): gate (sigmoid + top-8 + weight normalization), token->expert
dispatch, weight repacking in PE-friendly layout, final scatter-add
combine + LayerNorm.

Device (Bass/Tile, SPMD over 8 cores): 8 experts per core.  For each
expert the 4-layer MLP runs with *feature-major* activations
(act^T: [feat, tokens]) so that every matmul uses the natural-layout
weight tile [K=128, M=128] as the stationary operand and the activation
tile [K=128, T] as the moving operand -- no transposes anywhere.

HBM traffic is the roofline: w2 (the largest tensor, 50% of weight
bytes) is stored in fp8 e3m4 (4 mantissa bits) scaled by 128; the
dequant rides the Gelu activation's free `scale=` operand.  All other
weights stream as bf16.  Every DMA moves >=2KB-contiguous lines:
weights as 1-2MB megatiles, token/output tiles packed per expert slot
([128, 8C] / [128, 6C]).
"""

import numpy as np
import ml_dtypes

# problem constants (hardcoded; kernel.py must be self-contained)
B, S, D = 2, 512, 1024
H, BN, O = 2048, 256, 768
E, TOPK = 64, 8
N = B * S
NCORES = 8
EPC = E // NCORES  # experts per core

BF16 = ml_dtypes.bfloat16
E3M4 = ml_dtypes.float8_e3m4
W2_SCALE = 128.0  # power of two: dequant by psum * (1/128) is exact
W1_BLK = 512      # int8 block: one w1 row x 512 output-cols per scale

LAST_EXEC_NS = None  # test harness reads this after a traced run
LAST_RAW = None      # per-core raw device outputs (debug)


# ---------------------------------------------------------------------------
# host-side routing
# ---------------------------------------------------------------------------

def _route(x, gate_w, gate_bias):
    """Replicates the reference gate in float64: returns top_idx [N,8],
    combine weights wc [N,8] (float32)."""
    xf = x.reshape(N, D).astype(np.float64)
    logits = xf @ gate_w.astype(np.float64).T
    scores = 1.0 / (1.0 + np.exp(-logits))
    choice = scores + gate_bias.astype(np.float64)[None, :]
    # top-8, descending, stable (matches jax.lax.top_k tie behavior)
    top_idx = np.argsort(-choice, axis=1, kind="stable")[:, :TOPK]
    top_scores = np.take_along_axis(choice, top_idx, axis=1)
    wc = top_scores / (top_scores.sum(-1, keepdims=True) + 1e-6)
    return top_idx.astype(np.int64), wc.astype(np.float32)


def _assign_experts(counts):
    """Greedy balance: experts -> cores (EPC slots each), sorted desc within
    a core.  Returns assign[core][slot] = expert id."""
    order = np.argsort(-counts, kind="stable")
    loads = [0] * NCORES
    nslot = [0] * NCORES
    assign = [[] for _ in range(NCORES)]
    for e in order:
        c = min(
            (c for c in range(NCORES) if nslot[c] < EPC),
            key=lambda c: (loads[c], c),
        )
        assign[c].append(int(e))
        loads[c] += int(counts[e])
        nslot[c] += 1
    return assign  # each list already desc by count (greedy order)


# ---------------------------------------------------------------------------
# device program
# ---------------------------------------------------------------------------

def _build_program(caps):
    import concourse.bacc as bacc
    import concourse.tile as tile
    from concourse import mybir

    DT = mybir.dt.bfloat16
    F8 = mybir.dt.float8e3
    F32 = mybir.dt.float32
    SC = int(np.sum(caps))
    offs = np.concatenate([[0], np.cumsum(caps)]).astype(int)

    I8 = mybir.dt.int8
    nc = bacc.Bacc(trn_type="TRN2")
    w1s = nc.dram_tensor("w1s", [EPC, 2, 128, 8192], I8, kind="ExternalInput")
    w2s = nc.dram_tensor("w2s", [EPC, 4, 128, 8192], F8, kind="ExternalInput")
    # w3 ([128,4096]) and w4 ([128,1536]) fused into one DMA per expert
    w34s = nc.dram_tensor("w34s", [EPC, 128, 5632], DT, kind="ExternalInput")
    xt = nc.dram_tensor("xt", [128, 8 * SC], DT, kind="ExternalInput")
    bias = nc.dram_tensor("bias", [128, EPC * 40], F32, kind="ExternalInput")
    # w1 int8 dequant scales: col r*32 + (g*2+h)*8 + k (h = 512-col half)
    wscl = nc.dram_tensor("wscl", [128, EPC * 32], F32, kind="ExternalInput")
    out = nc.dram_tensor("out", [128, 6 * SC], DT, kind="ExternalOutput")

    GELU = mybir.ActivationFunctionType.Gelu
    INV_S = 1.0 / W2_SCALE

    with tile.TileContext(nc) as tc:
        with (
            tc.tile_pool(name="w1i", bufs=3) as w1ipool,
            tc.tile_pool(name="w1p", bufs=2) as w1pool,
            tc.tile_pool(name="w2p", bufs=6) as w2pool,
            tc.tile_pool(name="w34p", bufs=2) as w34pool,
            tc.tile_pool(name="xtp", bufs=3) as xpool,
            tc.tile_pool(name="h1p", bufs=32) as h1pool,
            tc.tile_pool(name="h2p", bufs=32) as h2pool,
            tc.tile_pool(name="h3p", bufs=4) as h3pool,
            tc.tile_pool(name="outp", bufs=3) as opool,
            tc.tile_pool(name="ps", bufs=8, space="PSUM") as pspool,
            tc.tile_pool(name="one", bufs=1) as single,
        ):
            bias_sb = single.tile([128, EPC * 40], F32)
            nc.sync.dma_start(out=bias_sb, in_=bias[:, :])
            wscl_sb = single.tile([128, EPC * 32], F32)
            nc.sync.dma_start(out=wscl_sb, in_=wscl[:, :])
            # Observer ops: ACT and DVE each touch the bias tile once so the
            # bias-DMA tick is already observed by those engines -- keeps every
            # later activation/tensor_scalar at <=1 sync wait (the legacy
            # walrus codegen rejects instructions with 2+ waits).
            obs_a = single.tile([128, 1], F32)
            nc.scalar.copy(out=obs_a, in_=bias_sb[:, 0:1])
            obs_v = single.tile([128, 1], F32)
            nc.vector.tensor_copy(out=obs_v, in_=bias_sb[:, 0:1])
            obs_s = single.tile([128, 1], F32)
            nc.vector.tensor_copy(out=obs_s, in_=wscl_sb[:, 0:1])
            obs_g = single.tile([128, 1], F32)
            nc.gpsimd.tensor_copy(out=obs_g, in_=wscl_sb[:, 0:1])

            for r in range(EPC):
                C = int(caps[r])
                off = int(offs[r])
                bcol = r * 40

                # gathered tokens, transposed+packed: [128, 8C]; k-tile k at
                # columns [k*C, (k+1)*C)
                xts = xpool.tile([128, 8 * C], DT, tag="xt")
                nc.sync.dma_start(
                    out=xts, in_=xt[:, 8 * off:8 * off + 8 * C]
                )

                # ---- L1: h1^T[H, C] = gelu(W1^T x + b1), K=D (8 tiles) ----
                # w1 streams as int8; DVE dequantizes to bf16 with per-
                # (row, 512-col half) scales before the PE reads it.
                h1 = []
                for g in range(2):  # m-groups of 8 feature tiles
                    wi = w1ipool.tile([128, 8192], I8, tag="w1i")
                    nc.sync.dma_start(out=wi, in_=w1s[r, g])
                    wt = w1pool.tile([128, 8192], DT, tag="w1")
                    for k in range(8):
                        # split dequant across DVE and GPSIMD (both idle-ish)
                        # so the dequant critical path stays ahead of the PE
                        eng = nc.vector if k % 2 == 0 else nc.gpsimd
                        for h in range(2):
                            scol = r * 32 + (g * 2 + h) * 8 + k
                            eng.tensor_scalar_mul(
                                wt[:, k * 1024 + h * 512: k * 1024 + (h + 1) * 512],
                                wi[:, k * 1024 + h * 512: k * 1024 + (h + 1) * 512],
                                wscl_sb[:, scol:scol + 1],
                            )
                    # m-pair-outer / k-inner: each PSUM bank pair finishes
                    # after 16 MMs and drains (gelu) while the PE streams on
                    # -- no group-boundary PE idle.
                    for mp in range(4):
                        ps = [pspool.tile([128, C], F32, tag="ps",
                                          name=f"ps1_{r}_{g}_{mp}_{i}")
                              for i in range(2)]
                        for k in range(8):
                            for i in range(2):
                                m = 2 * mp + i
                                nc.tensor.matmul(
                                    ps[i],
                                    wt[:, k * 1024 + m * 128: k * 1024 + (m + 1) * 128],
                                    xts[:, k * C:(k + 1) * C],
                                    start=(k == 0),
                                    stop=(k == 7),
                                )
                        for i in range(2):
                            m = 2 * mp + i
                            hh = h1pool.tile([128, C], DT, tag="h1")
                            nc.scalar.activation(
                                out=hh, in_=ps[i], func=GELU,
                                bias=bias_sb[:, bcol + g * 8 + m: bcol + g * 8 + m + 1],
                            )
                            h1.append(hh)

                # ---- L2: h2^T[H, C] = gelu((W2s^T h1)/s + b2), K=H ----
                h2 = []
                for g in range(2):
                    whs = []
                    for half in range(2):
                        wt = w2pool.tile([128, 8192], F8, tag="w2")
                        nc.sync.dma_start(out=wt, in_=w2s[r, g * 2 + half])
                        whs.append(wt)
                    for mp in range(4):
                        ps = [pspool.tile([128, C], F32, tag="ps",
                                          name=f"ps2_{r}_{g}_{mp}_{i}")
                              for i in range(2)]
                        for k in range(16):
                            wt = whs[k // 8]
                            c = k % 8
                            for i in range(2):
                                m = 2 * mp + i
                                nc.tensor.matmul(
                                    ps[i],
                                    wt[:, c * 1024 + m * 128: c * 1024 + (m + 1) * 128],
                                    h1[k],
                                    start=(k == 0),
                                    stop=(k == 15),
                                )
                        for i in range(2):
                            m = 2 * mp + i
                            hh = h2pool.tile([128, C], DT, tag="h2")
                            nc.scalar.activation(
                                out=hh, in_=ps[i], func=GELU, scale=INV_S,
                                bias=bias_sb[:, bcol + 16 + g * 8 + m: bcol + 16 + g * 8 + m + 1],
                            )
                            h2.append(hh)

                # ---- L3: h3^T[BN, C] = W3^T h2 + b3, K=H (16 tiles) ----
                wt34 = w34pool.tile([128, 5632], DT, tag="w34")
                nc.sync.dma_start(out=wt34, in_=w34s[r])
                psums3 = [pspool.tile([128, C], F32, tag="ps",
                                      name=f"ps3_{r}_{m}") for m in range(2)]
                for k in range(16):
                    for m in range(2):
                        nc.tensor.matmul(
                            psums3[m],
                            wt34[:, k * 256 + m * 128: k * 256 + (m + 1) * 128],
                            h2[k],
                            start=(k == 0),
                            stop=(k == 15),
                        )
                h3 = []
                for m in range(2):
                    hh = h3pool.tile([128, C], DT, tag="h3")
                    nc.vector.tensor_scalar_add(
                        hh, psums3[m], bias_sb[:, bcol + 32 + m: bcol + 32 + m + 1]
                    )
                    h3.append(hh)

                # ---- L4: out^T[O, C] = W4^T h3 + b4, K=BN (2 tiles) ----
                psums4 = [pspool.tile([128, C], F32, tag="ps",
                                      name=f"ps4_{r}_{m}") for m in range(6)]
                for c in range(2):
                    for m in range(6):
                        nc.tensor.matmul(
                            psums4[m],
                            wt34[:, 4096 + c * 768 + m * 128:
                                 4096 + c * 768 + (m + 1) * 128],
                            h3[c],
                            start=(c == 0),
                            stop=(c == 1),
                        )
                ot = opool.tile([128, 6 * C], DT, tag="out")
                for m in range(6):
                    nc.vector.tensor_scalar_add(
                        ot[:, m * C:(m + 1) * C], psums4[m],
                        bias_sb[:, bcol + 34 + m: bcol + 34 + m + 1]
                    )
                nc.sync.dma_start(
                    out=out[:, 6 * off:6 * off + 6 * C], in_=ot
                )

    # Official TRN2 sync legalization: moves extra matmul waits onto
    # LDWEIGHTS and splits any remaining multi-wait instructions via event
    # semaphores ("at most 1 wait per instruction" HW constraint).
    nc.compile()
    return nc


# ---------------------------------------------------------------------------
# host-side packing
# ---------------------------------------------------------------------------

def _pack_core(w1, b1, w2, b2, w3, b3, w4, b4, experts):
    """Pack one core's 8 experts into the DRAM layouts the program expects."""
    idx = np.asarray(experts)

    # W1 [e,1024,2048] -> int8 [EPC,2,128,8192] with per-(row, 512-col)
    # absmax scales.  Mega g holds k-chunks 0..7 of [128,1024] for m-group
    # g (chunk (g,k) = w1[e][k*128:(k+1)*128, g*1024:(g+1)*1024])
    w1sel = w1[idx]                                    # [EPC, 1024, 2048]
    blk = w1sel.reshape(EPC, 1024, 2048 // W1_BLK, W1_BLK)
    scl = np.abs(blk).max(axis=3) / 127.0              # [EPC, 1024, 4]
    scl[scl == 0] = 1.0
    q = np.clip(np.round(blk / scl[..., None]), -127, 127).astype(np.int8)
    q = q.reshape(EPC, 1024, 2048)
    a = q.reshape(EPC, 8, 128, 2, 1024)
    a = a.transpose(0, 3, 1, 2, 4)          # [EPC, g, k, 128, 1024]
    w1p = np.ascontiguousarray(
        a.transpose(0, 1, 3, 2, 4)          # [EPC, g, 128, k, 1024]
    ).reshape(EPC, 2, 128, 8192)
    # scales -> [128, EPC*32]: col r*32 + (g*2+h)*8 + k, row p = d k*128+p
    # scl[:, d, g*2+h] with d = k*128+p
    sclp = np.zeros((128, EPC * 32), np.float32)
    s4 = scl.reshape(EPC, 8, 128, 4)        # [r, k, p, g*2+h]
    for r in range(EPC):
        for gh in range(4):
            for k in range(8):
                sclp[:, r * 32 + gh * 8 + k] = s4[r, k, :, gh]

    # W2 [e,2048,2048] -> fp8 e3m4 * 128 -> [EPC,4,128,8192]: mega g*2+half
    # holds k-chunks half*8..half*8+7 for m-group g
    a = w2[idx].reshape(EPC, 16, 128, 2, 1024)
    a = a.transpose(0, 3, 1, 2, 4)          # [EPC, g, k16, 128, 1024]
    a = a.reshape(EPC, 2, 2, 8, 128, 1024)  # [EPC, g, half, c, 128, 1024]
    a = np.ascontiguousarray(
        a.transpose(0, 1, 2, 4, 3, 5)       # [EPC, g, half, 128, c, 1024]
    ).reshape(EPC, 4, 128, 8192)
    w2p = np.clip(a * np.float32(W2_SCALE), -15.5, 15.5).astype(E3M4)

    # W3 [e,2048,256] + W4 [e,256,768] fused -> [EPC,128,5632]:
    # cols 0..4096 = w3 k-chunks 0..15 of [128,256]; cols 4096..5632 = w4
    # k-chunks 0..1 of [128,768]
    a = w3[idx].reshape(EPC, 16, 128, 256)
    w3p = a.transpose(0, 2, 1, 3).reshape(EPC, 128, 4096)
    a = w4[idx].reshape(EPC, 2, 128, 768)
    w4p = a.transpose(0, 2, 1, 3).reshape(EPC, 128, 1536)
    w34p = np.ascontiguousarray(
        np.concatenate([w3p, w4p], axis=2)
    ).astype(BF16)

    # biases: per expert 40 cols of [128]: L1 m0-15 | L2 m0-15 | L3 m0-1 | L4 m0-5
    bb = np.concatenate(
        [
            b1[idx].reshape(EPC, 16, 128),
            b2[idx].reshape(EPC, 16, 128),
            b3[idx].reshape(EPC, 2, 128),
            b4[idx].reshape(EPC, 6, 128),
        ],
        axis=1,
    )  # [EPC, 40, 128]
    biasp = np.ascontiguousarray(
        bb.reshape(EPC * 40, 128).T
    ).astype(np.float32)  # [128, EPC*40]
    return w1p, w2p, w34p, biasp, sclp


def kernel(x, gate_w, gate_bias, w1, b1, w2, b2, w3, b3, w4, b4, ln_w, ln_b):
    global LAST_EXEC_NS, LAST_RAW
    x = np.asarray(x, np.float32)
    xf = x.reshape(N, D)

    top_idx, wc = _route(x, np.asarray(gate_w, np.float32),
                         np.asarray(gate_bias, np.float32))

    # token lists per expert
    counts = np.bincount(top_idx.ravel(), minlength=E)
    tok_of = [[] for _ in range(E)]
    w_of = [[] for _ in range(E)]
    flat_tok = np.repeat(np.arange(N), TOPK)
    flat_exp = top_idx.ravel()
    flat_w = wc.ravel()
    order = np.argsort(flat_exp, kind="stable")
    for t, e, w in zip(flat_tok[order], flat_exp[order], flat_w[order]):
        tok_of[e].append(int(t))
        w_of[e].append(float(w))

    assign = _assign_experts(counts)

    # per-slot capacities (shared across cores; slots sorted desc by count)
    caps = np.zeros(EPC, int)
    for c in range(NCORES):
        for r, e in enumerate(assign[c]):
            caps[r] = max(caps[r], counts[e])
    caps = ((caps + 1) // 2) * 2
    SC = int(caps.sum())
    offs = np.concatenate([[0], np.cumsum(caps)]).astype(int)

    nc = _build_program(caps)

    w1a = np.asarray(w1, np.float32); b1a = np.asarray(b1, np.float32)
    w2a = np.asarray(w2, np.float32); b2a = np.asarray(b2, np.float32)
    w3a = np.asarray(w3, np.float32); b3a = np.asarray(b3, np.float32)
    w4a = np.asarray(w4, np.float32); b4a = np.asarray(b4, np.float32)

    xt_bf = xf.T.astype(BF16)  # [D, N]
    in_maps = []
    for c in range(NCORES):
        w1p, w2p, w34p, biasp, sclp = _pack_core(
            w1a, b1a, w2a, b2a, w3a, b3a, w4a, b4a, assign[c]
        )
        # packed token tiles: [128, 8*SC]; slot r k-tile k at col
        # 8*offs[r] + k*caps[r], row p = feature k*128+p
        xtc = np.zeros((128, 8 * SC), BF16)
        for r, e in enumerate(assign[c]):
            ids = tok_of[e]
            base = 8 * offs[r]
            Cr = int(caps[r])
            if ids:
                g = xt_bf[:, ids].reshape(8, 128, len(ids))
                for k in range(8):
                    xtc[:, base + k * Cr: base + k * Cr + len(ids)] = g[k]
        in_maps.append(
            {"w1s": w1p, "w2s": w2p, "w34s": w34p,
             "xt": xtc, "bias": biasp, "wscl": sclp}
        )

    from concourse.bass_utils import run_bass_kernel_spmd

    res = run_bass_kernel_spmd(nc, in_maps, core_ids=list(range(NCORES)))
    LAST_EXEC_NS = res.exec_time_ns
    LAST_RAW = res.results

    # combine: scatter-add weighted expert outputs (float64 accum)
    combined = np.zeros((N, O), np.float64)
    for c in range(NCORES):
        yc = np.asarray(res.results[c]["out"]).astype(np.float32)  # [128, 6*SC] bf16
        for r, e in enumerate(assign[c]):
            ids = tok_of[e]
            if not ids:
                continue
            Cr = int(caps[r])
            base = 6 * offs[r]
            wv = np.asarray(w_of[e], np.float64)
            # y[m*128+p, t] = yc[p, base + m*Cr + t]
            y = yc[:, base:base + 6 * Cr].reshape(128, 6, Cr)[
                :, :, :len(ids)
            ].transpose(1, 0, 2).reshape(O, len(ids)).astype(np.float64)
            np.add.at(combined, ids, (y * wv[None, :]).T)

    combined = combined.astype(np.float32)
    mu = combined.mean(-1, keepdims=True)
    var = combined.var(-1, keepdims=True)
    outn = (combined - mu) / np.sqrt(var + 1e-5)
    outn = outn * np.asarray(ln_w, np.float32) + np.asarray(ln_b, np.float32)
    return outn.reshape(B, S, O).astype(np.float32)
